# revision 79
# baseline (speedup 1.0000x reference)
"""Trainium2 Bass kernel for the DGCNN layer (KNN-16 + edge MLP + max pool).

Sharding: data-parallel over batch (B=4) x query-halves (2 per batch) = 8
cores.  Each core holds all N=4096 points of its batch and computes KNN +
MLP + max for its 2048 queries.

Software-pipelined stages over 128-query tiles (emission order = engine
queue order, cross-engine deps >= 1 iteration old where possible):
  S1a: PE Gram s = 2*xi.xj - |xj|^2 via bf16 triple-split (x = h+m+l, 6
       cross pairs, K=21 contraction -> fp32-grade s at 1 cycle/row);
       ACT evicts, DMA spill of s.
  S1b: DVE chunk-max + top-NSEL chunk selection (Max8).  Points are
       Morton-sorted on the host so index-chunks are spatially coherent:
       5 chunks of 32 (160 candidates) almost always cover the 16 NNs.
  S1b2: NSEL per-chunk indirect gathers (one offset/partition each — the
       only indirect-DMA shape this hardware supports).
  S2:  DVE exact top-16 of candidates; flags: coverage gap (16th candidate
       vs (NSEL+1)-th chunk max) and tie gap (16th vs 17th candidate);
       jhi = cid[pw>>4] via the mask-sum trick.
  S3:  global ids j; 16 per-neighbor coord gathers + direct xi load into
       interleaved 8-wide slots; DMA-transpose puts coords on partitions.
  S4a: PE MLP1 (ONE K<=127 matmul per neighbor per H-half via staircase
       weights); ACT silu+b1 (b-half packed onto all 128 partitions).
  S4b: PE MLP2 (512-wide, 3-bank rotation); ACT f16 evicts.
  S4c: DVE TT-max trees (f16 2x mode) for the 16-neighbor max pool;
       single merged f16 store of all 3 C-blocks.

Queries whose coverage or tie gap is below fp32 Gram noise are flagged on
device and recomputed on host with reference-identical arithmetic (~9%
of queries at CHUNK=32/NSEL=5; the repair is exact, so correctness never
depends on the Morton heuristic).
"""

from contextlib import ExitStack

import numpy as np

import concourse.bacc as bacc
import concourse.bass as bass
import concourse.mybir as mybir
import concourse.tile as tile
from concourse.bass import IndirectOffsetOnAxis
from concourse.bass_utils import run_bass_kernel_spmd

F32 = mybir.dt.float32
F32R = mybir.dt.float32r
F16 = mybir.dt.float16
BF16 = mybir.dt.bfloat16
U16 = mybir.dt.uint16
U32 = mybir.dt.uint32
U64 = mybir.dt.uint64

# bf16 triple-split Gram: x = h+m+l (bf16 each); keep the 6 largest cross
# products (h,h),(h,m),(h,l),(m,h),(m,m),(l,h) per coordinate -> error
# ~2^-27, i.e. fp32-grade s at bf16 matmul speed (1 cycle/row vs 4).
SPLIT_PAIRS = [(0, 0), (0, 1), (0, 2), (1, 0), (1, 1), (2, 0)]
KGRAM = 3 * len(SPLIT_PAIRS) + 3  # 18 coord rows + 3 norm rows

B = 4
N = 4096
Q = 2048          # queries per core
NT = Q // 128     # query tiles per core
K = 16
H = 192
C = 384
CHUNK = 32
NCHUNK = N // CHUNK
NSEL = 5          # chunks gathered per query (Morton-coherent)
CSH = 5           # log2(CHUNK)

TIE_EPS = 2e-5    # flag threshold on s-gap (abs); s noise is ~1e-6

Alu = mybir.AluOpType
Act = mybir.ActivationFunctionType
Axis = mybir.AxisListType

# max-pool unit engine assignment per (hh*2+sub, c):
# 0 = DVE reduce from PSUM, 1 = ACT f16 evict + DVE TT tree.
# (Pool/GPSIMD cannot run TensorScalarPtr/TensorTensor per walrus codegen.)
MP_KIND = [1, 1, 1,
           1, 1, 1,
           1, 1, 1,
           1, 1, 1]


def build_program(nc: bass.Bass, nt=NT):
    qrs = nc.dram_tensor("qrs", [KGRAM, Q + N], BF16, kind="ExternalInput")
    pt4 = nc.dram_tensor("pt4", [N, 4], F16, kind="ExternalInput")
    # staircase weights: w1stk[8w:8w+7, w*H:(w+1)*H] = W1' (zeros elsewhere)
    # so MLP1 lhsT/rhs can both start at base partition 0 for every w.
    w1stk = nc.dram_tensor("w1stk", [128, K * H], F16, kind="ExternalInput")
    w2 = nc.dram_tensor("w2", [H, C], F16, kind="ExternalInput")
    b1d = nc.dram_tensor("b1d", [H, 1], F32, kind="ExternalInput")
    iota_rb = nc.dram_tensor("iota_rb", [128, K], U32, kind="ExternalInput")
    iota16 = nc.dram_tensor("iota16", [128, K], U32, kind="ExternalInput")
    iotap = nc.dram_tensor("iotap", [128, K], U32, kind="ExternalInput")
    xi_dram = nc.dram_tensor("xi_dram", [Q, 4], F16, kind="ExternalInput")

    outT = nc.dram_tensor("outT", [C, Q], F16, kind="ExternalOutput")
    flags = nc.dram_tensor("flags", [128, 2 * NT], F32, kind="ExternalOutput")

    s_dram = nc.dram_tensor("s_dram", [Q, N], F32, kind="Internal")
    # 2-D AP with indirection on axis=1 => coef = 1 (flat element offsets)
    s_flat = s_dram[:, :]

    with tile.TileContext(nc) as tc, ExitStack() as ctx:
        pers = ctx.enter_context(tc.tile_pool(name="pers", bufs=1))
        sp_scp = ctx.enter_context(tc.tile_pool(name="scp", bufs=3))
        sp_m = ctx.enter_context(tc.tile_pool(name="m", bufs=2))
        sp_cand = ctx.enter_context(tc.tile_pool(name="cand", bufs=3))
        sp_small = ctx.enter_context(tc.tile_pool(name="small", bufs=3))
        sp_xj = ctx.enter_context(tc.tile_pool(name="xj", bufs=3))
        sp_s1 = ctx.enter_context(tc.tile_pool(name="s1", bufs=4))
        sp_e2 = ctx.enter_context(tc.tile_pool(name="e2", bufs=3))
        sp_mp = ctx.enter_context(tc.tile_pool(name="mp", bufs=2))
        pspers = ctx.enter_context(
            tc.tile_pool(name="pspers", bufs=1, space=bass.MemorySpace.PSUM))

        qrs_sb = pers.tile([KGRAM, Q + N], BF16)
        qT = qrs_sb[:, 0:Q]
        rhsg = qrs_sb[:, Q:Q + N]
        w1sb = pers.tile([128, K * H], F16)
        w2a = pers.tile([128, C], F16)
        # w2 rows 128:192 duplicated on both partition halves so MLP2-b's
        # lhsT base partition can match its rhs (s1b packs w8 0-3 on
        # partitions 0-63 and w8 4-7 on 64-127).
        w2bb = pers.tile([128, C], F16)
        b1a = pers.tile([128, 1], F32)
        b1bb = pers.tile([128, 1], F32)
        io_rb = pers.tile([128, K], U32)   # p * N
        io_16 = pers.tile([128, K], U32)   # p * 16
        io_p = pers.tile([128, K], U32)    # p
        flg = pers.tile([128, 2 * NT], F32)
        nc.gpsimd.memset(flg[:, :], 0.0)

        # static PSUM: gram 2-bank ping, MLP1 (2+1 banks), MLP2 3-bank ping
        pg = [pspers.tile([128, 512], F32, name=f"pg{i}", tag=f"pg{i}")
              for i in range(2)]
        p1a = pspers.tile([128, 1024], F32, tag="p1a")
        p1b = pspers.tile([128, 512], F32, tag="p1b")
        p2 = [pspers.tile([128, 512], F32, name=f"p2_{i}", tag=f"p2_{i}")
              for i in range(3)]

        nc.sync.dma_start(qrs_sb[:, :], qrs[:, :])
        nc.sync.dma_start(w1sb[:, :], w1stk[:, :])
        nc.sync.dma_start(w2a[:, :], w2[0:128, :])
        nc.sync.dma_start(w2bb[0:64, :], w2[128:H, :])
        nc.sync.dma_start(w2bb[64:128, :], w2[128:H, :])
        nc.sync.dma_start(b1a[:, :], b1d[0:128, :])
        nc.sync.dma_start(b1bb[0:64, :], b1d[128:H, :])
        nc.sync.dma_start(b1bb[64:128, :], b1d[128:H, :])
        nc.sync.dma_start(io_rb[:, :], iota_rb[:, :])
        nc.sync.dma_start(io_16[:, :], iota16[:, :])
        nc.sync.dma_start(io_p[:, :], iotap[:, :])

        # setup dummies: absorb each input-load semaphore on the PE before
        # the pipelined matmuls so per-tile matmuls keep few sync waits.
        setup_dum = [
            (qrs_sb[0:2, 0:2], qrs_sb[0:2, 0:2]),
            (w1sb[0:2, 0:2], w1sb[0:2, 0:2]),
            (w2a[0:2, 0:2], w2a[0:2, 0:2]),
            (w2bb[0:2, 0:2], w2bb[0:2, 0:2]),
        ]
        for lhs_d, rhs_d in setup_dum:
            nc.tensor.matmul(pg[0][0:lhs_d.free_size(), 0:rhs_d.free_size()],
                             lhs_d, rhs_d,
                             start=True, stop=True, skip_group_check=True)

        st = {}  # per-tile cross-stage tiles

        def s1a(t):
            qs = t * 128
            scp = sp_scp.tile([128, N], F32, tag="scp")
            # tiny ACT op claims the scp slot (absorbs waits on the previous
            # occupant's readers) so the real evicts keep within 2 waits.
            nc.scalar.copy(scp[0:1, 0:1], scp[0:1, 0:1])
            for h in range(8):
                bank = pg[h % 2]
                nc.tensor.matmul(
                    bank[:, :],
                    qT[:, qs:qs + 128],
                    rhsg[:, h * 512:(h + 1) * 512],
                    start=True, stop=True, skip_group_check=True)
                nc.scalar.copy(scp[:, h * 512:(h + 1) * 512], bank[:, :])
                if h % 2 == 1:
                    nc.sync.dma_start(
                        s_dram[qs:qs + 128, (h - 1) * 512:(h + 1) * 512],
                        scp[:, (h - 1) * 512:(h + 1) * 512])
            st[t] = {"scp": scp}

        def s1b(t):
            qs = t * 128
            scp = st[t].pop("scp")

            # chunk-max
            M = sp_m.tile([128, NCHUNK], F32, tag="M")
            nc.vector.reduce_max(
                M[:, :],
                scp[:, :].rearrange("p (c w) -> p c w", w=CHUNK),
                axis=Axis.X)

            # L2: top-8 chunks.  Points are Morton-sorted on the host, so
            # chunks are spatially coherent and the 16 nearest neighbors
            # almost always live inside the best 8 chunks; the provable
            # coverage check (16th candidate value vs 9th chunk max) is
            # flagged in S2 and failures go to the exact host repair.
            m8 = sp_small.tile([128, 8], F32, tag="m8")
            cid8 = sp_small.tile([128, 8], U32, tag="cid8")
            nc.vector.max(m8[:, :], M[:, :])
            nc.vector.max_index(cid8[:, :], m8[:, :], M[:, :])
            cid = cid8[:, 0:NSEL]
            # Max8 output is sorted desc, so the (NSEL+1)-th chunk max --
            # the coverage bound -- is already m8[:, NSEL].
            st[t]["cid"] = cid
            st[t]["cm9"] = m8[:, NSEL:NSEL + 1]
            coff = sp_small.tile([128, NSEL], U32, tag="coff")
            nc.vector.tensor_scalar(
                coff[:, :], cid[:, :], CSH, None, Alu.logical_shift_left)
            nc.vector.tensor_tensor(
                coff[:, :], coff[:, :], io_rb[:, 0:NSEL], Alu.add)
            st[t]["coff"] = coff

        def s1b2(t):
            qs = t * 128
            coff = st[t].pop("coff")
            cand = sp_cand.tile([128, NSEL, CHUNK], F32, tag="cand")
            for w in range(NSEL):
                nc.gpsimd.indirect_dma_start(
                    cand[:, w, :], None, s_flat,
                    IndirectOffsetOnAxis(ap=coff[:, w:w + 1], axis=1),
                    element_offset=qs * N)
            st[t]["cand"] = cand

        def s2(t):
            qs = t * 128
            cand = st[t].pop("cand")
            cd = cand[:, :, :].rearrange("p a b -> p (a b)")
            v1 = sp_small.tile([128, 8], F32, tag="v1")
            v2 = sp_small.tile([128, 8], F32, tag="v2")
            pw = sp_small.tile([128, K], U32, tag="pw")
            nc.vector.max(v1[:, :], cd)
            nc.vector.max_index(pw[:, 0:8], v1[:, :], cd)
            nc.vector.match_replace(cd, v1[:, :], cd, -3.0e38)
            nc.vector.max(v2[:, :], cd)
            nc.vector.max_index(pw[:, 8:16], v2[:, :], cd)
            v17 = sp_small.tile([128, 1], F32, tag="v17")
            nc.vector.match_replace(cd, v2[:, :], cd, -3.0e38)
            nc.vector.reduce_max(v17[:, :], cd, axis=Axis.X)
            nc.vector.tensor_tensor(
                flg[:, 2 * t + 1:2 * t + 2], v2[:, 7:8], v17[:, :], Alu.subtract)
            # chunk-coverage flag: 16th candidate must beat the 9th chunk max
            cm9 = st[t].pop("cm9")
            nc.vector.tensor_tensor(
                flg[:, 2 * t:2 * t + 1], v2[:, 7:8], cm9[:, :], Alu.subtract)

            # j = cid[pw>>4]<<4 | (pw&15) via mask-sum (per-partition
            # scalar broadcast); cross-partition gathers can't do this.
            ow = sp_small.tile([128, K], U32, tag="ow")
            rw = sp_small.tile([128, K], U32, tag="rw")
            nc.vector.tensor_scalar(
                ow[:, :], pw[:, :], CHUNK - 1, None, Alu.bitwise_and)
            nc.vector.tensor_scalar(
                rw[:, :], pw[:, :], CSH, None, Alu.logical_shift_right)
            cid = st[t].pop("cid")
            jhi = sp_small.tile([128, K], U32, tag="jhi")
            jtmp = sp_small.tile([128, K], U32, tag="jtmp")
            nc.vector.tensor_scalar(jhi[:, :], rw[:, :], 0, None, Alu.mult)
            for r in range(NSEL):
                cbr = cid[:, r:r + 1].broadcast_to([128, K])
                nc.vector.scalar_tensor_tensor(
                    jtmp[:, :], rw[:, :], r, cbr,
                    Alu.is_equal, Alu.mult)
                nc.vector.tensor_tensor(
                    jhi[:, :], jhi[:, :], jtmp[:, :], Alu.add)
            st[t]["jhi"] = jhi
            st[t]["ow"] = ow

        def s3(t):
            qs = t * 128
            jhi = st[t].pop("jhi")
            ow = st[t].pop("ow")
            j32 = sp_small.tile([128, K], U32, tag="j32")
            nc.vector.tensor_scalar(
                j32[:, :], jhi[:, :], CSH, None, Alu.logical_shift_left)
            nc.vector.tensor_tensor(
                j32[:, :], j32[:, :], ow[:, :], Alu.bitwise_or)


            xjq = sp_xj.tile([128, 2 * K, 4], F16, tag="xjq")
            for w in range(K):
                nc.gpsimd.indirect_dma_start(
                    xjq[:, 2 * w + 1, :], None, pt4[:, :],
                    IndirectOffsetOnAxis(ap=j32[:, w:w + 1], axis=0),
                    element_offset=0)
            # xi: direct load of this tile's own coords (per-core input),
            # broadcast into the 16 even slots
            xi4 = sp_small.tile([128, 4], F16, tag="xi4")
            nc.sync.dma_start(xi4[:, :], xi_dram[qs:qs + 128, :])
            nc.vector.tensor_copy(
                xjq[:, :, :].rearrange("p (w two) c -> p w two c", two=2)
                [:, :, 0, :],
                xi4[:, :].rearrange("p (o c) -> p o c", o=1)
                .broadcast_to([128, K, 4]))
            xjT = sp_xj.tile([128, 128], F16, tag="xjT")
            nc.sync.dma_start_transpose(
                xjT[:, :], xjq[:, :, :].rearrange("p a b -> p (a b)"))
            st[t]["xjT"] = xjT

        def s4a(t):
            xjT = st[t].pop("xjT")
            sil = []
            for hh in range(2):
                for w8 in range(8):
                    w = hh * 8 + w8
                    kk = 8 * w + 7
                    nc.tensor.matmul(
                        p1a[:, w8 * 128:(w8 + 1) * 128],
                        w1sb[0:kk, w * H:w * H + 128],
                        xjT[0:kk, :], start=True, stop=True,
                        skip_group_check=True)
                    # b-half packed: w8 0-3 on partitions 0-63, 4-7 on 64-127
                    pb = 64 * (w8 >> 2)
                    nc.tensor.matmul(
                        p1b[pb:pb + 64, (w8 & 3) * 128:(w8 & 3) * 128 + 128],
                        w1sb[0:kk, w * H + 128:(w + 1) * H],
                        xjT[0:kk, :], start=True, stop=True,
                        skip_group_check=True)
                sa = sp_s1.tile([128, 1024], F16, tag="s1a")
                sb = sp_s1.tile([128, 512], F16, tag="s1b")
                nc.scalar.activation(sa[:, :], p1a[:, :], Act.Silu,
                                     bias=b1a[:, :])
                nc.scalar.activation(sb[:, :], p1b[:, :], Act.Silu,
                                     bias=b1bb[:, :])
                sil.append((sa, sb))
            st[t]["sil"] = sil

        def s4b(t):
            sil = st[t].pop("sil")
            # t2all[:, u, :] = per-unit 4-neighbor max; kind-0 units reduce
            # straight from PSUM here, others evict f16 for the tree stage.
            t2all = sp_mp.tile([128, 12, 128], F16, tag="t2all")
            e2s = {}
            pp = 0
            for hh in range(2):
                s1a, s1b = sil[hh]
                for sub in range(2):
                    ss = sub * 512
                    for c in range(3):
                        u = (hh * 2 + sub) * 3 + c
                        bank = p2[pp % 3]
                        pp += 1
                        nc.tensor.matmul(
                            bank[:, :], w2a[:, c * 128:(c + 1) * 128],
                            s1a[:, ss:ss + 512], start=True, stop=False,
                            skip_group_check=True)
                        nc.tensor.matmul(
                            bank[:, :],
                            w2bb[64 * sub:64 * sub + 64,
                                 c * 128:(c + 1) * 128],
                            s1b[64 * sub:64 * sub + 64, :],
                            start=False, stop=True,
                            skip_group_check=True)
                        if MP_KIND[u] == 0:
                            nc.vector.reduce_max(
                                t2all[:, u, :],
                                bank[:, :].rearrange("p (n q) -> p q n",
                                                     q=128),
                                axis=Axis.X)
                        else:
                            e2 = sp_e2.tile([128, 512], F16, tag=f"e2_{u}")
                            nc.scalar.copy(e2[:, :], bank[:, :])
                            e2s[u] = e2
            st[t]["t2all"] = t2all
            st[t]["e2s"] = e2s

        def s4c(t):
            qs = t * 128
            d = st.pop(t)
            t2all = d["t2all"]
            e2s = d["e2s"]
            # finish the deferred trees (f16; DVE 2x mode or Pool stt)
            for u, e2 in e2s.items():
                th = sp_mp.tile([128, 256], F16, tag="th")
                nc.vector.tensor_tensor(
                    th[:, :], e2[:, 0:256], e2[:, 256:512], Alu.max)
                nc.vector.tensor_tensor(
                    t2all[:, u, :], th[:, 0:128], th[:, 128:256],
                    Alu.max)
            # combine the 4 row-groups per c and store all 3 c-blocks at once
            acc = sp_mp.tile([128, 3, 128], F16, tag="acc")
            for c in range(3):
                x = sp_mp.tile([128, 128], F16, tag="cmb")
                nc.vector.tensor_tensor(
                    x[:, :], t2all[:, c, :], t2all[:, 3 + c, :], Alu.max)
                nc.vector.tensor_tensor(
                    acc[:, c, :], t2all[:, 6 + c, :], t2all[:, 9 + c, :],
                    Alu.max)
                nc.vector.tensor_tensor(
                    acc[:, c, :], acc[:, c, :], x[:, :], Alu.max)
            nc.sync.dma_start(
                outT[:, qs:qs + 128].rearrange("(c p) q -> p c q", p=128),
                acc[:, :, :])

        # Emission order within an iteration = desired engine-queue order:
        # the S3 gather->transpose chain first (its inputs are a full
        # iteration old), then S2, then S1, then S4.
        for i in range(nt + 5):
            if 0 <= i - 5 < nt:
                s4c(i - 5)
            if 0 <= i - 3 < nt:
                s3(i - 3)
            if 0 <= i - 1 < nt:
                s1b2(i - 1)
            if 0 <= i - 2 < nt:
                s2(i - 2)
            if i < nt:
                s1a(i)
                s1b(i)
            if 0 <= i - 4 < nt:
                s4a(i - 4)
                s4b(i - 4)

        nc.sync.dma_start(flags[:, :], flg[:, :])

    return nc


def _morton_perm(p):
    """Spatial sort so index-contiguous chunks are spatially coherent."""
    f = p.astype(np.float64)
    lo = f.min(0)
    g = ((f - lo) / (f.max(0) - lo + 1e-12) * 1023.0).astype(np.uint64)
    code = np.zeros(len(g), np.uint64)
    for bit in range(10):
        for d in range(3):
            code |= ((g[:, d] >> np.uint64(bit)) & np.uint64(1)) <<                 np.uint64(3 * bit + d)
    return np.argsort(code, kind="stable")


def host_inputs_for_core(core, point, W1, b1, W2, b2):
    b = core // 2
    half = core % 2
    p0 = np.asarray(point[b], dtype=np.float32)
    p = p0[_morton_perm(p0)]
    qsl = slice(half * Q, (half + 1) * Q)
    w1a = np.asarray(W1[:3], np.float32)
    w1b = np.asarray(W1[3:], np.float32)
    w1c7 = np.concatenate([w1b - w1a, np.zeros((1, H), np.float32), w1a], 0)
    w1stk = np.zeros((128, K, H), np.float16)
    for w in range(K):
        w1stk[8 * w:8 * w + 7, w, :] = w1c7.astype(np.float16)

    # bf16 triple-split Gram operands (see SPLIT_PAIRS in build_program)
    import ml_dtypes
    bf16 = ml_dtypes.bfloat16

    def split3(v):
        v = v.astype(np.float64)
        h = v.astype(bf16)
        r = v - h.astype(np.float64)
        m = r.astype(bf16)
        l = (r - m.astype(np.float64)).astype(bf16)
        return [h, m, l]

    a = split3(p[qsl].T)                       # xi (raw), [3][3, 1024*2]
    bsp = split3((2.0 * p.astype(np.float64)).T)   # 2*xj
    nsp = split3(-(p.astype(np.float64) ** 2).sum(-1)[None, :])
    qrs = np.zeros((KGRAM, Q + N), bf16)
    r = 0
    for c in range(3):
        for ia, ib in SPLIT_PAIRS:
            qrs[r, :Q] = a[ia][c]
            qrs[r, Q:] = bsp[ib][c]
            r += 1
    for j in range(3):
        qrs[r, :Q] = bf16(1.0)
        qrs[r, Q:] = nsp[j][0]
        r += 1
    return {
        "pt4": np.ascontiguousarray(np.concatenate(
            [p.astype(np.float16), np.zeros((N, 1), np.float16)], 1)),
        "xi_dram": np.ascontiguousarray(np.concatenate(
            [p[qsl].astype(np.float16), np.zeros((Q, 1), np.float16)], 1)),
        "qrs": np.ascontiguousarray(qrs),
        "w1stk": np.ascontiguousarray(w1stk.reshape(128, K * H)),
        "w2": np.ascontiguousarray(np.asarray(W2, np.float16)),
        "b1d": np.ascontiguousarray(np.asarray(b1, np.float32)[:, None]),
        "iota_rb": np.ascontiguousarray(
            (np.arange(128, dtype=np.uint32)[:, None] * np.uint32(N))
            * np.ones((1, K), np.uint32)),
        "iota16": np.ascontiguousarray(
            (np.arange(128, dtype=np.uint32)[:, None] * np.uint32(K))
            * np.ones((1, K), np.uint32)),
        # per-core: global row of this core's query p is half*Q + qs + p
        "iotap": np.ascontiguousarray(
            (np.arange(128, dtype=np.uint32)[:, None]
             + np.uint32(half * Q))
            * np.ones((1, K), np.uint32)),

    }


def _host_repair(out, flags_per_core, point, W1, b1, W2, b2, k, perms):
    """Recompute flagged (possibly tie-ambiguous) queries with
    reference-identical fp32 arithmetic."""
    f32 = np.float32
    W1 = np.asarray(W1, f32)
    b1 = np.asarray(b1, f32)
    W2 = np.asarray(W2, f32)
    b2 = np.asarray(b2, f32)
    n_repaired = 0
    for core in range(2 * B):
        b = core // 2
        half = core % 2
        fl = flags_per_core[core].reshape(128, NT, 2)
        gap = fl.min(-1)                        # [128, NT]
        pp, tt = np.nonzero(gap < TIE_EPS)
        if len(pp) == 0:
            continue
        qidx = perms[b][half * Q + tt * 128 + pp]
        pb = np.asarray(point[b], f32)
        diff = pb[qidx][:, None, :] - pb[None, :, :]
        dist = (diff * diff).sum(-1)
        idx = np.argsort(dist, axis=-1, kind="stable")[:, :k]
        neigh = pb[idx]
        rel = neigh - pb[qidx][:, None, :]
        ctr = np.broadcast_to(pb[qidx][:, None, :], rel.shape)
        feat = np.concatenate([rel, ctr], -1)
        h = feat @ W1 + b1
        h = h * (f32(1.0) / (f32(1.0) + np.exp(-h)))
        h2 = h @ W2 + b2
        out[b, qidx, :] = h2.max(-2)
        n_repaired += len(pp)
    return n_repaired


_CACHE = {}


def _get_program():
    if "nc" not in _CACHE:
        nc = bacc.Bacc("TRN2", debug=False, num_swdge_queues=1)
        build_program(nc)
        nc.compile()
        _CACHE["nc"] = nc
    return _CACHE["nc"]


def kernel(point, W1, b1, W2, b2, k, _trace=False):
    point = np.asarray(point, np.float32)
    k = int(k)
    assert k == K and point.shape == (B, N, 3)

    nc = _get_program()
    perms = [_morton_perm(np.asarray(point[b], np.float32))
             for b in range(B)]
    in_maps = [host_inputs_for_core(c, point, W1, b1, W2, b2)
               for c in range(2 * B)]
    try:
        res = run_bass_kernel_spmd(nc, in_maps, core_ids=list(range(2 * B)),
                                   trace=_trace)
    except ModuleNotFoundError:
        res = run_bass_kernel_spmd(nc, in_maps, core_ids=list(range(2 * B)),
                                   trace=False)

    out = np.empty((B, N, C), np.float32)
    flags_per_core = []
    for core in range(2 * B):
        b = core // 2
        half = core % 2
        outT = np.asarray(res.results[core]["outT"])
        out[b, perms[b][half * Q:(half + 1) * Q], :] = \
            outT.T.astype(np.float32)
        flags_per_core.append(np.asarray(res.results[core]["flags"]))
    out += np.asarray(b2, np.float32)[None, None, :]

    n_rep = _host_repair(out, flags_per_core, point, W1, b1, W2, b2, k,
                         perms)
    if _trace:
        return out, res, n_rep
    return out


# revision 80
# speedup vs baseline: 1.0397x; 1.0397x over previous
"""Trainium2 Bass kernel for the DGCNN layer (KNN-16 + edge MLP + max pool).

Sharding: data-parallel over batch (B=4) x query-halves (2 per batch) = 8
cores.  Each core holds all N=4096 points of its batch and computes KNN +
MLP + max for its 2048 queries.

Software-pipelined stages over 128-query tiles (emission order = engine
queue order, cross-engine deps >= 1 iteration old where possible):
  S1a: PE Gram s = 2*xi.xj - |xj|^2 via bf16 triple-split (x = h+m+l, 6
       cross pairs, K=21 contraction -> fp32-grade s at 1 cycle/row);
       ACT evicts, DMA spill of s.
  S1b: DVE chunk-max + top-NSEL chunk selection (Max8).  Points are
       Morton-sorted on the host so index-chunks are spatially coherent:
       the NSEL*CHUNK candidates almost always cover the 16 true NNs.
  S1b2: NSEL per-chunk indirect gathers (one offset/partition each — the
       only indirect-DMA shape this hardware supports).
  S2:  DVE exact top-16 of candidates; flags: coverage gap (16th candidate
       vs (NSEL+1)-th chunk max) and tie gap (16th vs 17th candidate);
       jhi = cid[pw>>4] via the mask-sum trick.
  S3:  global ids j; 16 per-neighbor coord gathers + direct xi load into
       interleaved 8-wide slots; DMA-transpose puts coords on partitions.
  S4a: PE MLP1 (ONE K<=127 matmul per neighbor per H-half via staircase
       weights); ACT silu+b1 (b-half packed onto all 128 partitions).
  S4b: PE MLP2 (512-wide, 3-bank rotation); ACT f16 evicts.
  S4c: DVE TT-max trees (f16 2x mode) for the 16-neighbor max pool;
       single merged f16 store of all 3 C-blocks.

Queries whose coverage or tie gap is below fp32 Gram noise are flagged on
device and recomputed on host with reference-identical arithmetic (~10%
of queries; the repair is exact, so correctness never depends on the
Morton heuristic).
"""

from contextlib import ExitStack

import numpy as np

import concourse.bacc as bacc
import concourse.bass as bass
import concourse.mybir as mybir
import concourse.tile as tile
from concourse.bass import IndirectOffsetOnAxis
from concourse.bass_utils import run_bass_kernel_spmd

F32 = mybir.dt.float32
F32R = mybir.dt.float32r
F16 = mybir.dt.float16
BF16 = mybir.dt.bfloat16
U16 = mybir.dt.uint16
U32 = mybir.dt.uint32
U64 = mybir.dt.uint64

# bf16 triple-split Gram: x = h+m+l (bf16 each); keep the 6 largest cross
# products (h,h),(h,m),(h,l),(m,h),(m,m),(l,h) per coordinate -> error
# ~2^-27, i.e. fp32-grade s at bf16 matmul speed (1 cycle/row vs 4).
SPLIT_PAIRS = [(0, 0), (0, 1), (0, 2), (1, 0), (1, 1), (2, 0)]
KGRAM = 3 * len(SPLIT_PAIRS) + 3  # 18 coord rows + 3 norm rows

B = 4
N = 4096
Q = 2048          # queries per core
NT = Q // 128     # query tiles per core
K = 16
H = 192
C = 384
CHUNK = 64
NCHUNK = N // CHUNK
NSEL = 4          # chunks gathered per query (Morton-coherent)
CSH = 6           # log2(CHUNK)

TIE_EPS = 2e-5    # flag threshold on s-gap (abs); s noise is ~1e-6

Alu = mybir.AluOpType
Act = mybir.ActivationFunctionType
Axis = mybir.AxisListType

# max-pool unit engine assignment per (hh*2+sub, c):
# 0 = DVE reduce from PSUM, 1 = ACT f16 evict + DVE TT tree.
# (Pool/GPSIMD cannot run TensorScalarPtr/TensorTensor per walrus codegen.)
MP_KIND = [1, 1, 1,
           1, 1, 1,
           1, 1, 1,
           1, 1, 1]


def build_program(nc: bass.Bass, nt=NT):
    qrs = nc.dram_tensor("qrs", [KGRAM, Q + N], BF16, kind="ExternalInput")
    pt4 = nc.dram_tensor("pt4", [N, 4], F16, kind="ExternalInput")
    # staircase weights: w1stk[8w:8w+7, w*H:(w+1)*H] = W1' (zeros elsewhere)
    # so MLP1 lhsT/rhs can both start at base partition 0 for every w.
    w1stk = nc.dram_tensor("w1stk", [128, K * H], F16, kind="ExternalInput")
    w2 = nc.dram_tensor("w2", [H, C], F16, kind="ExternalInput")
    b1d = nc.dram_tensor("b1d", [H, 1], F32, kind="ExternalInput")
    iota_rb = nc.dram_tensor("iota_rb", [128, K], U32, kind="ExternalInput")
    iota16 = nc.dram_tensor("iota16", [128, K], U32, kind="ExternalInput")
    iotap = nc.dram_tensor("iotap", [128, K], U32, kind="ExternalInput")
    xi_dram = nc.dram_tensor("xi_dram", [Q, 4], F16, kind="ExternalInput")

    outT = nc.dram_tensor("outT", [C, Q], F16, kind="ExternalOutput")
    flags = nc.dram_tensor("flags", [128, 2 * NT], F32, kind="ExternalOutput")

    s_dram = nc.dram_tensor("s_dram", [Q, N], F32, kind="Internal")
    # 2-D AP with indirection on axis=1 => coef = 1 (flat element offsets)
    s_flat = s_dram[:, :]

    with tile.TileContext(nc) as tc, ExitStack() as ctx:
        pers = ctx.enter_context(tc.tile_pool(name="pers", bufs=1))
        sp_scp = ctx.enter_context(tc.tile_pool(name="scp", bufs=3))
        sp_m = ctx.enter_context(tc.tile_pool(name="m", bufs=2))
        sp_cand = ctx.enter_context(tc.tile_pool(name="cand", bufs=3))
        sp_small = ctx.enter_context(tc.tile_pool(name="small", bufs=3))
        sp_xj = ctx.enter_context(tc.tile_pool(name="xj", bufs=3))
        sp_s1 = ctx.enter_context(tc.tile_pool(name="s1", bufs=4))
        sp_e2 = ctx.enter_context(tc.tile_pool(name="e2", bufs=3))
        sp_mp = ctx.enter_context(tc.tile_pool(name="mp", bufs=2))
        pspers = ctx.enter_context(
            tc.tile_pool(name="pspers", bufs=1, space=bass.MemorySpace.PSUM))

        qrs_sb = pers.tile([KGRAM, Q + N], BF16)
        qT = qrs_sb[:, 0:Q]
        rhsg = qrs_sb[:, Q:Q + N]
        w1sb = pers.tile([128, K * H], F16)
        w2a = pers.tile([128, C], F16)
        # w2 rows 128:192 duplicated on both partition halves so MLP2-b's
        # lhsT base partition can match its rhs (s1b packs w8 0-3 on
        # partitions 0-63 and w8 4-7 on 64-127).
        w2bb = pers.tile([128, C], F16)
        b1a = pers.tile([128, 1], F32)
        b1bb = pers.tile([128, 1], F32)
        io_rb = pers.tile([128, K], U32)   # p * N
        io_16 = pers.tile([128, K], U32)   # p * 16
        io_p = pers.tile([128, K], U32)    # p
        flg = pers.tile([128, 2 * NT], F32)
        nc.gpsimd.memset(flg[:, :], 0.0)

        # static PSUM: gram 2-bank ping, MLP1 (2+1 banks), MLP2 3-bank ping
        pg = [pspers.tile([128, 512], F32, name=f"pg{i}", tag=f"pg{i}")
              for i in range(2)]
        p1a = pspers.tile([128, 1024], F32, tag="p1a")
        p1b = pspers.tile([128, 512], F32, tag="p1b")
        p2 = [pspers.tile([128, 512], F32, name=f"p2_{i}", tag=f"p2_{i}")
              for i in range(3)]

        nc.sync.dma_start(qrs_sb[:, :], qrs[:, :])
        nc.sync.dma_start(w1sb[:, :], w1stk[:, :])
        nc.sync.dma_start(w2a[:, :], w2[0:128, :])
        nc.sync.dma_start(w2bb[0:64, :], w2[128:H, :])
        nc.sync.dma_start(w2bb[64:128, :], w2[128:H, :])
        nc.sync.dma_start(b1a[:, :], b1d[0:128, :])
        nc.sync.dma_start(b1bb[0:64, :], b1d[128:H, :])
        nc.sync.dma_start(b1bb[64:128, :], b1d[128:H, :])
        nc.sync.dma_start(io_rb[:, :], iota_rb[:, :])
        nc.sync.dma_start(io_16[:, :], iota16[:, :])
        nc.sync.dma_start(io_p[:, :], iotap[:, :])

        # setup dummies: absorb each input-load semaphore on the PE before
        # the pipelined matmuls so per-tile matmuls keep few sync waits.
        setup_dum = [
            (qrs_sb[0:2, 0:2], qrs_sb[0:2, 0:2]),
            (w1sb[0:2, 0:2], w1sb[0:2, 0:2]),
            (w2a[0:2, 0:2], w2a[0:2, 0:2]),
            (w2bb[0:2, 0:2], w2bb[0:2, 0:2]),
        ]
        for lhs_d, rhs_d in setup_dum:
            nc.tensor.matmul(pg[0][0:lhs_d.free_size(), 0:rhs_d.free_size()],
                             lhs_d, rhs_d,
                             start=True, stop=True, skip_group_check=True)

        st = {}  # per-tile cross-stage tiles

        def s1a(t):
            qs = t * 128
            scp = sp_scp.tile([128, N], F32, tag="scp")
            # tiny ACT op claims the scp slot (absorbs waits on the previous
            # occupant's readers) so the real evicts keep within 2 waits.
            nc.scalar.copy(scp[0:1, 0:1], scp[0:1, 0:1])
            for h in range(8):
                bank = pg[h % 2]
                nc.tensor.matmul(
                    bank[:, :],
                    qT[:, qs:qs + 128],
                    rhsg[:, h * 512:(h + 1) * 512],
                    start=True, stop=True, skip_group_check=True)
                nc.scalar.copy(scp[:, h * 512:(h + 1) * 512], bank[:, :])
                if h % 2 == 1:
                    nc.sync.dma_start(
                        s_dram[qs:qs + 128, (h - 1) * 512:(h + 1) * 512],
                        scp[:, (h - 1) * 512:(h + 1) * 512])
            st[t] = {"scp": scp}

        def s1b(t):
            qs = t * 128
            scp = st[t].pop("scp")

            # chunk-max
            M = sp_m.tile([128, NCHUNK], F32, tag="M")
            nc.vector.reduce_max(
                M[:, :],
                scp[:, :].rearrange("p (c w) -> p c w", w=CHUNK),
                axis=Axis.X)

            # L2: top-8 chunks.  Points are Morton-sorted on the host, so
            # chunks are spatially coherent and the 16 nearest neighbors
            # almost always live inside the best 8 chunks; the provable
            # coverage check (16th candidate value vs 9th chunk max) is
            # flagged in S2 and failures go to the exact host repair.
            m8 = sp_small.tile([128, 8], F32, tag="m8")
            cid8 = sp_small.tile([128, 8], U32, tag="cid8")
            nc.vector.max(m8[:, :], M[:, :])
            nc.vector.max_index(cid8[:, :], m8[:, :], M[:, :])
            cid = cid8[:, 0:NSEL]
            # Max8 output is sorted desc, so the (NSEL+1)-th chunk max --
            # the coverage bound -- is already m8[:, NSEL].
            st[t]["cid"] = cid
            st[t]["cm9"] = m8[:, NSEL:NSEL + 1]
            coff = sp_small.tile([128, NSEL], U32, tag="coff")
            nc.vector.tensor_scalar(
                coff[:, :], cid[:, :], CSH, None, Alu.logical_shift_left)
            nc.vector.tensor_tensor(
                coff[:, :], coff[:, :], io_rb[:, 0:NSEL], Alu.add)
            st[t]["coff"] = coff

        def s1b2(t):
            qs = t * 128
            coff = st[t].pop("coff")
            cand = sp_cand.tile([128, NSEL, CHUNK], F32, tag="cand")
            for w in range(NSEL):
                nc.gpsimd.indirect_dma_start(
                    cand[:, w, :], None, s_flat,
                    IndirectOffsetOnAxis(ap=coff[:, w:w + 1], axis=1),
                    element_offset=qs * N)
            st[t]["cand"] = cand

        def s2(t):
            qs = t * 128
            cand = st[t].pop("cand")
            cd = cand[:, :, :].rearrange("p a b -> p (a b)")
            v1 = sp_small.tile([128, 8], F32, tag="v1")
            v2 = sp_small.tile([128, 8], F32, tag="v2")
            pw = sp_small.tile([128, K], U32, tag="pw")
            nc.vector.max(v1[:, :], cd)
            nc.vector.max_index(pw[:, 0:8], v1[:, :], cd)
            nc.vector.match_replace(cd, v1[:, :], cd, -3.0e38)
            nc.vector.max(v2[:, :], cd)
            nc.vector.max_index(pw[:, 8:16], v2[:, :], cd)
            v17 = sp_small.tile([128, 1], F32, tag="v17")
            nc.vector.match_replace(cd, v2[:, :], cd, -3.0e38)
            nc.vector.reduce_max(v17[:, :], cd, axis=Axis.X)
            nc.vector.tensor_tensor(
                flg[:, 2 * t + 1:2 * t + 2], v2[:, 7:8], v17[:, :], Alu.subtract)
            # chunk-coverage flag: 16th candidate must beat the 9th chunk max
            cm9 = st[t].pop("cm9")
            nc.vector.tensor_tensor(
                flg[:, 2 * t:2 * t + 1], v2[:, 7:8], cm9[:, :], Alu.subtract)

            # j = cid[pw>>4]<<4 | (pw&15) via mask-sum (per-partition
            # scalar broadcast); cross-partition gathers can't do this.
            ow = sp_small.tile([128, K], U32, tag="ow")
            rw = sp_small.tile([128, K], U32, tag="rw")
            nc.vector.tensor_scalar(
                ow[:, :], pw[:, :], CHUNK - 1, None, Alu.bitwise_and)
            nc.vector.tensor_scalar(
                rw[:, :], pw[:, :], CSH, None, Alu.logical_shift_right)
            cid = st[t].pop("cid")
            jhi = sp_small.tile([128, K], U32, tag="jhi")
            jtmp = sp_small.tile([128, K], U32, tag="jtmp")
            nc.vector.tensor_scalar(jhi[:, :], rw[:, :], 0, None, Alu.mult)
            for r in range(NSEL):
                cbr = cid[:, r:r + 1].broadcast_to([128, K])
                nc.vector.scalar_tensor_tensor(
                    jtmp[:, :], rw[:, :], r, cbr,
                    Alu.is_equal, Alu.mult)
                nc.vector.tensor_tensor(
                    jhi[:, :], jhi[:, :], jtmp[:, :], Alu.add)
            st[t]["jhi"] = jhi
            st[t]["ow"] = ow

        def s3(t):
            qs = t * 128
            jhi = st[t].pop("jhi")
            ow = st[t].pop("ow")
            j32 = sp_small.tile([128, K], U32, tag="j32")
            nc.vector.tensor_scalar(
                j32[:, :], jhi[:, :], CSH, None, Alu.logical_shift_left)
            nc.vector.tensor_tensor(
                j32[:, :], j32[:, :], ow[:, :], Alu.bitwise_or)


            xjq = sp_xj.tile([128, 2 * K, 4], F16, tag="xjq")
            for w in range(K):
                nc.gpsimd.indirect_dma_start(
                    xjq[:, 2 * w + 1, :], None, pt4[:, :],
                    IndirectOffsetOnAxis(ap=j32[:, w:w + 1], axis=0),
                    element_offset=0)
            # xi: direct load of this tile's own coords (per-core input),
            # broadcast into the 16 even slots
            xi4 = sp_small.tile([128, 4], F16, tag="xi4")
            nc.sync.dma_start(xi4[:, :], xi_dram[qs:qs + 128, :])
            nc.vector.tensor_copy(
                xjq[:, :, :].rearrange("p (w two) c -> p w two c", two=2)
                [:, :, 0, :],
                xi4[:, :].rearrange("p (o c) -> p o c", o=1)
                .broadcast_to([128, K, 4]))
            xjT = sp_xj.tile([128, 128], F16, tag="xjT")
            nc.sync.dma_start_transpose(
                xjT[:, :], xjq[:, :, :].rearrange("p a b -> p (a b)"))
            st[t]["xjT"] = xjT

        def s4a(t):
            xjT = st[t].pop("xjT")
            sil = []
            for hh in range(2):
                for w8 in range(8):
                    w = hh * 8 + w8
                    kk = 8 * w + 7
                    nc.tensor.matmul(
                        p1a[:, w8 * 128:(w8 + 1) * 128],
                        w1sb[0:kk, w * H:w * H + 128],
                        xjT[0:kk, :], start=True, stop=True,
                        skip_group_check=True)
                    # b-half packed: w8 0-3 on partitions 0-63, 4-7 on 64-127
                    pb = 64 * (w8 >> 2)
                    nc.tensor.matmul(
                        p1b[pb:pb + 64, (w8 & 3) * 128:(w8 & 3) * 128 + 128],
                        w1sb[0:kk, w * H + 128:(w + 1) * H],
                        xjT[0:kk, :], start=True, stop=True,
                        skip_group_check=True)
                sa = sp_s1.tile([128, 1024], F16, tag="s1a")
                sb = sp_s1.tile([128, 512], F16, tag="s1b")
                nc.scalar.activation(sa[:, :], p1a[:, :], Act.Silu,
                                     bias=b1a[:, :])
                nc.scalar.activation(sb[:, :], p1b[:, :], Act.Silu,
                                     bias=b1bb[:, :])
                sil.append((sa, sb))
            st[t]["sil"] = sil

        def s4b(t):
            sil = st[t].pop("sil")
            # t2all[:, u, :] = per-unit 4-neighbor max; kind-0 units reduce
            # straight from PSUM here, others evict f16 for the tree stage.
            t2all = sp_mp.tile([128, 12, 128], F16, tag="t2all")
            e2s = {}
            pp = 0
            for hh in range(2):
                s1a, s1b = sil[hh]
                for sub in range(2):
                    ss = sub * 512
                    for c in range(3):
                        u = (hh * 2 + sub) * 3 + c
                        bank = p2[pp % 3]
                        pp += 1
                        nc.tensor.matmul(
                            bank[:, :], w2a[:, c * 128:(c + 1) * 128],
                            s1a[:, ss:ss + 512], start=True, stop=False,
                            skip_group_check=True)
                        nc.tensor.matmul(
                            bank[:, :],
                            w2bb[64 * sub:64 * sub + 64,
                                 c * 128:(c + 1) * 128],
                            s1b[64 * sub:64 * sub + 64, :],
                            start=False, stop=True,
                            skip_group_check=True)
                        if MP_KIND[u] == 0:
                            nc.vector.reduce_max(
                                t2all[:, u, :],
                                bank[:, :].rearrange("p (n q) -> p q n",
                                                     q=128),
                                axis=Axis.X)
                        else:
                            e2 = sp_e2.tile([128, 512], F16, tag=f"e2_{u}")
                            nc.scalar.copy(e2[:, :], bank[:, :])
                            e2s[u] = e2
            st[t]["t2all"] = t2all
            st[t]["e2s"] = e2s

        def s4c(t):
            qs = t * 128
            d = st.pop(t)
            t2all = d["t2all"]
            e2s = d["e2s"]
            # finish the deferred trees (f16; DVE 2x mode or Pool stt)
            for u, e2 in e2s.items():
                th = sp_mp.tile([128, 256], F16, tag="th")
                nc.vector.tensor_tensor(
                    th[:, :], e2[:, 0:256], e2[:, 256:512], Alu.max)
                nc.vector.tensor_tensor(
                    t2all[:, u, :], th[:, 0:128], th[:, 128:256],
                    Alu.max)
            # combine the 4 row-groups per c and store all 3 c-blocks at once
            acc = sp_mp.tile([128, 3, 128], F16, tag="acc")
            for c in range(3):
                x = sp_mp.tile([128, 128], F16, tag="cmb")
                nc.vector.tensor_tensor(
                    x[:, :], t2all[:, c, :], t2all[:, 3 + c, :], Alu.max)
                nc.vector.tensor_tensor(
                    acc[:, c, :], t2all[:, 6 + c, :], t2all[:, 9 + c, :],
                    Alu.max)
                nc.vector.tensor_tensor(
                    acc[:, c, :], acc[:, c, :], x[:, :], Alu.max)
            nc.sync.dma_start(
                outT[:, qs:qs + 128].rearrange("(c p) q -> p c q", p=128),
                acc[:, :, :])

        # Emission order within an iteration = desired engine-queue order:
        # the S3 gather->transpose chain first (its inputs are a full
        # iteration old), then S2, then S1, then S4.
        for i in range(nt + 5):
            if 0 <= i - 5 < nt:
                s4c(i - 5)
            if 0 <= i - 3 < nt:
                s3(i - 3)
            if 0 <= i - 1 < nt:
                s1b2(i - 1)
            if 0 <= i - 2 < nt:
                s2(i - 2)
            if i < nt:
                s1a(i)
                s1b(i)
            if 0 <= i - 4 < nt:
                s4a(i - 4)
                s4b(i - 4)

        nc.sync.dma_start(flags[:, :], flg[:, :])

    return nc


def _morton_perm(p):
    """Spatial sort so index-contiguous chunks are spatially coherent."""
    f = p.astype(np.float64)
    lo = f.min(0)
    g = ((f - lo) / (f.max(0) - lo + 1e-12) * 1023.0).astype(np.uint64)
    code = np.zeros(len(g), np.uint64)
    for bit in range(10):
        for d in range(3):
            code |= ((g[:, d] >> np.uint64(bit)) & np.uint64(1)) <<                 np.uint64(3 * bit + d)
    return np.argsort(code, kind="stable")


def host_inputs_for_core(core, point, W1, b1, W2, b2):
    b = core // 2
    half = core % 2
    p0 = np.asarray(point[b], dtype=np.float32)
    p = p0[_morton_perm(p0)]
    qsl = slice(half * Q, (half + 1) * Q)
    w1a = np.asarray(W1[:3], np.float32)
    w1b = np.asarray(W1[3:], np.float32)
    w1c7 = np.concatenate([w1b - w1a, np.zeros((1, H), np.float32), w1a], 0)
    w1stk = np.zeros((128, K, H), np.float16)
    for w in range(K):
        w1stk[8 * w:8 * w + 7, w, :] = w1c7.astype(np.float16)

    # bf16 triple-split Gram operands (see SPLIT_PAIRS in build_program)
    import ml_dtypes
    bf16 = ml_dtypes.bfloat16

    def split3(v):
        v = v.astype(np.float64)
        h = v.astype(bf16)
        r = v - h.astype(np.float64)
        m = r.astype(bf16)
        l = (r - m.astype(np.float64)).astype(bf16)
        return [h, m, l]

    a = split3(p[qsl].T)                       # xi (raw), [3][3, 1024*2]
    bsp = split3((2.0 * p.astype(np.float64)).T)   # 2*xj
    nsp = split3(-(p.astype(np.float64) ** 2).sum(-1)[None, :])
    qrs = np.zeros((KGRAM, Q + N), bf16)
    r = 0
    for c in range(3):
        for ia, ib in SPLIT_PAIRS:
            qrs[r, :Q] = a[ia][c]
            qrs[r, Q:] = bsp[ib][c]
            r += 1
    for j in range(3):
        qrs[r, :Q] = bf16(1.0)
        qrs[r, Q:] = nsp[j][0]
        r += 1
    return {
        "pt4": np.ascontiguousarray(np.concatenate(
            [p.astype(np.float16), np.zeros((N, 1), np.float16)], 1)),
        "xi_dram": np.ascontiguousarray(np.concatenate(
            [p[qsl].astype(np.float16), np.zeros((Q, 1), np.float16)], 1)),
        "qrs": np.ascontiguousarray(qrs),
        "w1stk": np.ascontiguousarray(w1stk.reshape(128, K * H)),
        "w2": np.ascontiguousarray(np.asarray(W2, np.float16)),
        "b1d": np.ascontiguousarray(np.asarray(b1, np.float32)[:, None]),
        "iota_rb": np.ascontiguousarray(
            (np.arange(128, dtype=np.uint32)[:, None] * np.uint32(N))
            * np.ones((1, K), np.uint32)),
        "iota16": np.ascontiguousarray(
            (np.arange(128, dtype=np.uint32)[:, None] * np.uint32(K))
            * np.ones((1, K), np.uint32)),
        # per-core: global row of this core's query p is half*Q + qs + p
        "iotap": np.ascontiguousarray(
            (np.arange(128, dtype=np.uint32)[:, None]
             + np.uint32(half * Q))
            * np.ones((1, K), np.uint32)),

    }


def _host_repair(out, flags_per_core, point, W1, b1, W2, b2, k, perms):
    """Recompute flagged (possibly tie-ambiguous) queries with
    reference-identical fp32 arithmetic."""
    f32 = np.float32
    W1 = np.asarray(W1, f32)
    b1 = np.asarray(b1, f32)
    W2 = np.asarray(W2, f32)
    b2 = np.asarray(b2, f32)
    n_repaired = 0
    for core in range(2 * B):
        b = core // 2
        half = core % 2
        fl = flags_per_core[core].reshape(128, NT, 2)
        gap = fl.min(-1)                        # [128, NT]
        pp, tt = np.nonzero(gap < TIE_EPS)
        if len(pp) == 0:
            continue
        qidx = perms[b][half * Q + tt * 128 + pp]
        pb = np.asarray(point[b], f32)
        diff = pb[qidx][:, None, :] - pb[None, :, :]
        dist = (diff * diff).sum(-1)
        idx = np.argsort(dist, axis=-1, kind="stable")[:, :k]
        neigh = pb[idx]
        rel = neigh - pb[qidx][:, None, :]
        ctr = np.broadcast_to(pb[qidx][:, None, :], rel.shape)
        feat = np.concatenate([rel, ctr], -1)
        h = feat @ W1 + b1
        h = h * (f32(1.0) / (f32(1.0) + np.exp(-h)))
        h2 = h @ W2 + b2
        out[b, qidx, :] = h2.max(-2)
        n_repaired += len(pp)
    return n_repaired


_CACHE = {}


def _get_program():
    if "nc" not in _CACHE:
        nc = bacc.Bacc("TRN2", debug=False, num_swdge_queues=1)
        build_program(nc)
        nc.compile()
        _CACHE["nc"] = nc
    return _CACHE["nc"]


def kernel(point, W1, b1, W2, b2, k, _trace=False):
    point = np.asarray(point, np.float32)
    k = int(k)
    assert k == K and point.shape == (B, N, 3)

    nc = _get_program()
    perms = [_morton_perm(np.asarray(point[b], np.float32))
             for b in range(B)]
    in_maps = [host_inputs_for_core(c, point, W1, b1, W2, b2)
               for c in range(2 * B)]
    try:
        res = run_bass_kernel_spmd(nc, in_maps, core_ids=list(range(2 * B)),
                                   trace=_trace)
    except ModuleNotFoundError:
        res = run_bass_kernel_spmd(nc, in_maps, core_ids=list(range(2 * B)),
                                   trace=False)

    out = np.empty((B, N, C), np.float32)
    flags_per_core = []
    for core in range(2 * B):
        b = core // 2
        half = core % 2
        outT = np.asarray(res.results[core]["outT"])
        out[b, perms[b][half * Q:(half + 1) * Q], :] = \
            outT.T.astype(np.float32)
        flags_per_core.append(np.asarray(res.results[core]["flags"]))
    out += np.asarray(b2, np.float32)[None, None, :]

    n_rep = _host_repair(out, flags_per_core, point, W1, b1, W2, b2, k,
                         perms)
    if _trace:
        return out, res, n_rep
    return out
